# revision 9
# baseline (speedup 1.0000x reference)
"""GCNConv(flow=target_to_source) + BatchNorm + ReLU + residual, on 8 trn2 NeuronCores.

Math: with self-loops appended to the edge list,
    deg[i]   = #{e : row[e] == i}
    dinv     = deg ** -0.5
    v        = dinv[:, None] * x                      (bf16 table in DRAM)
    S[i]     = sum_{e: row[e]=i} v[col[e]]            (dma_gather + onehot-matmul scatter)
    out      = dinv[:, None] * (S @ W)                (W commutes past the aggregation)
    y        = relu((out - mean) * rsqrt(var + eps) * gamma + beta) + x
(b cancels inside BatchNorm, so it is dropped.)

Sharding: nodes (rows) are split across 8 cores; edges are partitioned by
destination row so the scatter-add is core-local PSUM accumulation.  Each core
builds the full v table locally; BN statistics go through a [128,2] AllReduce.

dma_gather takes int16 indices, so the v table is addressed as two halves
(lo: rows < SPLIT, hi: rows >= SPLIT) and each block's edges are ordered
lo-cols-first.  Index buffers are packed in the HW layout: idx i at
(partition i%16, column i//16), replicated across the eight 16-partition
groups.
"""

import os
import sys

sys.path.insert(0, "/opt/trn_rl_repo")
os.environ.setdefault("MYCRO_LOCAL_CACHE", "1")

from contextlib import ExitStack

import ml_dtypes
import numpy as np

CORES = 8
BN_EPS = 1e-5
SPLIT = 32768
_CACHE: dict = {}


def _pick_blk(npc: int) -> int:
    for blk in range(125, 0, -1):
        if npc % blk == 0:
            return blk
    raise ValueError(npc)


def _strided(ap_src, offset_elems, dims):
    import concourse.bass as bass

    return bass.AP(ap_src.tensor, offset_elems, [list(d) for d in dims])


def _build_nc(N, D, NPC, BLK, NBLK, NVT, VB, t_lo, t_hi, SUP, SPL,
              XF_PAD, XL_PAD, RPF_PAD, RPL_PAD):
    from concourse import bacc, bass, mybir, tile
    from concourse.masks import make_identity

    f32 = mybir.dt.float32
    bf16 = mybir.dt.bfloat16
    i16 = mybir.dt.int16
    T = t_lo + t_hi

    nc = bacc.Bacc(
        "TRN2",
        target_bir_lowering=False,
        debug=False,
        enable_asserts=False,
        num_devices=CORES,
    )

    x_bf = nc.dram_tensor("x_bf", [XF_PAD, D], bf16, kind="ExternalInput").ap()
    rp_full = nc.dram_tensor("rp_full", [RPF_PAD], f32, kind="ExternalInput").ap()
    rp_loc = nc.dram_tensor("rp_loc", [RPL_PAD], f32, kind="ExternalInput").ap()
    lo_t = nc.dram_tensor("lo_idx", [128, NBLK * t_lo * 8], i16, kind="ExternalInput").ap()
    hi_t = nc.dram_tensor("hi_idx", [128, NBLK * t_hi * 8], i16, kind="ExternalInput").ap()
    rel_t = nc.dram_tensor("rel_arr", [128, NBLK * T], bf16, kind="ExternalInput").ap()
    xloc_t = nc.dram_tensor("x_loc", [XL_PAD, D], f32, kind="ExternalInput").ap()
    w_t = nc.dram_tensor("w_mat", [D, D], f32, kind="ExternalInput").ap()
    gamma_t = nc.dram_tensor("gamma", [D], f32, kind="ExternalInput").ap()
    beta_t = nc.dram_tensor("beta", [D], f32, kind="ExternalInput").ap()
    iota_t = nc.dram_tensor("iota", [128, BLK], bf16, kind="ExternalInput").ap()
    y_t = nc.dram_tensor("y_out", [NPC, D], f32, kind="ExternalOutput").ap()

    with tile.TileContext(nc) as tc, ExitStack() as ctx:
        const = ctx.enter_context(tc.tile_pool(name="const", bufs=1))
        vstage = ctx.enter_context(tc.tile_pool(name="vstage", bufs=3))
        gath = ctx.enter_context(tc.tile_pool(name="gath", bufs=3))
        ohp = ctx.enter_context(tc.tile_pool(name="ohp", bufs=3))
        evp = ctx.enter_context(tc.tile_pool(name="evp", bufs=2))
        big = ctx.enter_context(tc.tile_pool(name="big", bufs=1))
        ps_main = ctx.enter_context(tc.tile_pool(name="ps_main", bufs=2, space="PSUM"))
        ps_stat = ctx.enter_context(tc.tile_pool(name="ps_stat", bufs=1, space="PSUM"))
        ps_misc = ctx.enter_context(tc.tile_pool(name="ps_misc", bufs=2, space="PSUM"))
        dram = ctx.enter_context(tc.tile_pool(name="dram", bufs=1, space="DRAM"))

        # ---- constants -----------------------------------------------------
        w_sb = const.tile([D, D], f32)
        nc.sync.dma_start(w_sb[:], w_t[:])
        iota_sb = const.tile([128, BLK], bf16)
        nc.sync.dma_start(iota_sb[:], iota_t[:])
        lo_sb = const.tile([128, NBLK * t_lo * 8], i16)
        nc.sync.dma_start(lo_sb[:], lo_t[:])
        hi_sb = const.tile([128, NBLK * t_hi * 8], i16)
        nc.sync.dma_start(hi_sb[:], hi_t[:])
        rel_sb = const.tile([128, NBLK * T], bf16)
        nc.sync.dma_start(rel_sb[:], rel_t[:])
        ones_sb = const.tile([128, 1], f32)
        nc.vector.memset(ones_sb[:], 1.0)
        onesrow_sb = const.tile([1, 128], f32)
        nc.vector.memset(onesrow_sb[:], 1.0)
        gb_sb = const.tile([128, 2], f32)
        nc.sync.dma_start(gb_sb[:, 0:1], gamma_t[:, None])
        nc.sync.dma_start(gb_sb[:, 1:2], beta_t[:, None])
        ident_sb = const.tile([128, 128], f32)
        make_identity(nc, ident_sb[:])

        # ---- dinv ----------------------------------------------------------
        def build_dinv(rp_ap, nt):
            rpa = vstage.tile([128, nt], f32, tag="rpa")
            rpb = vstage.tile([128, nt], f32, tag="rpb")
            nc.sync.dma_start(rpa[:], _strided(rp_ap, 0, [[1, 128], [BLK, nt]]))
            nc.sync.dma_start(rpb[:], _strided(rp_ap, 1, [[1, 128], [BLK, nt]]))
            deg = vstage.tile([128, nt], f32, tag="deg")
            nc.vector.tensor_tensor(
                out=deg[:BLK, :], in0=rpb[:BLK, :], in1=rpa[:BLK, :],
                op=mybir.AluOpType.subtract,
            )
            rec = vstage.tile([128, nt], f32, tag="rec")
            nc.vector.reciprocal(out=rec[:BLK, :], in_=deg[:BLK, :])
            dv = const.tile([128, nt], f32, name=f"dinv{nt}")
            nc.scalar.sqrt(out=dv[:BLK, :], in_=rec[:BLK, :])
            return dv

        dinvf = build_dinv(rp_full, NVT)
        dinvl = build_dinv(rp_loc, NBLK)

        # ---- v table: v = dinv * x (bf16, in DRAM) -------------------------
        v_dram = dram.tile([XF_PAD, D], bf16)
        zt = const.tile([128, D], bf16, name="zt")
        nc.vector.memset(zt[:], 0.0)
        nc.sync.dma_start(v_dram[N:XF_PAD, :], zt[: XF_PAD - N, :])
        for vt in range(NVT // VB):
            xs = vstage.tile([128, VB * D], bf16, tag="xs")
            nc.sync.dma_start(
                xs[:BLK, :],
                _strided(x_bf, vt * VB * BLK * D, [[D, BLK], [BLK * D, VB], [1, D]]),
            )
            vv = vstage.tile([128, VB * D], bf16, tag="vv")
            dinv_rep = _strided(
                dinvf[:], vt * VB, [[dinvf[:].ap[0][0], BLK], [1, VB], [0, D]]
            )
            nc.vector.tensor_tensor(
                out=vv[:BLK, :], in0=xs[:BLK, :], in1=dinv_rep,
                op=mybir.AluOpType.mult,
            )
            nc.sync.dma_start(
                _strided(v_dram[:], vt * VB * BLK * D, [[D, BLK], [BLK * D, VB], [1, D]]),
                vv[:BLK, :],
            )

        # ---- main loop: SUP blocks per gather chunk ------------------------
        out_all = big.tile([128, NBLK * D], f32)
        s1 = ps_stat.tile([128, 1], f32, tag="s1")
        s2 = ps_stat.tile([128, 1], f32, tag="s2")
        for c0 in range(0, NBLK, SUP):
            g = gath.tile([128, SUP * T, D], bf16)
            nc.gpsimd.dma_gather(
                g[:, 0:SUP * t_lo, :],
                v_dram[0:SPL, :],
                lo_sb[:, c0 * t_lo * 8:(c0 + SUP) * t_lo * 8],
                SUP * t_lo * 128,
                SUP * t_lo * 128,
                D,
                single_packet=False,
            )
            nc.gpsimd.dma_gather(
                g[:, SUP * t_lo:SUP * T, :],
                v_dram[SPL:XF_PAD, :],
                hi_sb[:, c0 * t_hi * 8:(c0 + SUP) * t_hi * 8],
                SUP * t_hi * 128,
                SUP * t_hi * 128,
                D,
                single_packet=False,
            )
            for j in range(SUP):
                blk = c0 + j
                oh = ohp.tile([128, T, BLK], bf16)
                iota_rep = _strided(
                    iota_sb[:], 0, [list(iota_sb[:].ap[0]), [0, T], [1, BLK]]
                )
                rel_rep = _strided(
                    rel_sb[:], blk * T, [list(rel_sb[:].ap[0]), [1, T], [0, BLK]]
                )
                nc.vector.tensor_tensor(
                    out=oh[:], in0=iota_rep, in1=rel_rep, op=mybir.AluOpType.is_equal
                )
                st = ps_main.tile([128, BLK], f32, tag="st")
                for t in range(T):
                    if t < t_lo:
                        src = g[:, j * t_lo + t, :]
                    else:
                        src = g[:, SUP * t_lo + j * t_hi + (t - t_lo), :]
                    nc.tensor.matmul(
                        out=st[:], lhsT=src, rhs=oh[:, t, :],
                        start=(t == 0), stop=(t == T - 1),
                    )
                stb = evp.tile([128, BLK], f32, tag="stb")
                nc.vector.tensor_copy(out=stb[:], in_=st[:])
                ow = ps_main.tile([BLK, D], f32, tag="ow")
                nc.tensor.matmul(out=ow[:], lhsT=stb[:], rhs=w_sb[:], start=True, stop=True)
                oslice = out_all[:BLK, blk * D:(blk + 1) * D]
                nc.vector.tensor_scalar(
                    out=oslice, in0=ow[:], scalar1=dinvl[:BLK, blk:blk + 1],
                    scalar2=None, op0=mybir.AluOpType.mult,
                )
                sq_s = evp.tile([128, D], f32, tag="sq")
                nc.vector.tensor_tensor(
                    out=sq_s[:BLK, :], in0=oslice, in1=oslice, op=mybir.AluOpType.mult
                )
                nc.tensor.matmul(
                    out=s1[:], lhsT=oslice, rhs=ones_sb[:BLK, :],
                    start=(blk == 0), stop=(blk == NBLK - 1),
                )
                nc.tensor.matmul(
                    out=s2[:], lhsT=sq_s[:BLK, :], rhs=ones_sb[:BLK, :],
                    start=(blk == 0), stop=(blk == NBLK - 1),
                )

        # ---- BN stats AllReduce + affine params ----------------------------
        stat_sb = const.tile([128, 2], f32, name="stat_sb")
        nc.vector.tensor_copy(out=stat_sb[:, 0:1], in_=s1[:])
        nc.vector.tensor_copy(out=stat_sb[:, 1:2], in_=s2[:])
        cc_in = dram.tile([128, 2], f32)
        cc_out = dram.tile([128, 2], f32, addr_space="Shared")
        nc.sync.dma_start(cc_in[:], stat_sb[:])
        nc.gpsimd.collective_compute(
            "AllReduce",
            mybir.AluOpType.add,
            replica_groups=[list(range(CORES))],
            ins=[cc_in.opt()],
            outs=[cc_out.opt()],
        )
        statg = const.tile([128, 2], f32, name="statg")
        nc.sync.dma_start(statg[:], cc_out[:])

        invn = 1.0 / float(N)
        mean = const.tile([128, 1], f32, name="mean")
        nc.vector.tensor_scalar(
            out=mean[:], in0=statg[:, 0:1], scalar1=invn, scalar2=None,
            op0=mybir.AluOpType.mult,
        )
        vareps = const.tile([128, 1], f32, name="vareps")
        m2 = const.tile([128, 1], f32, name="m2")
        nc.vector.tensor_tensor(out=m2[:], in0=mean[:], in1=mean[:], op=mybir.AluOpType.mult)
        nc.vector.tensor_scalar(
            out=vareps[:], in0=statg[:, 1:2], scalar1=invn, scalar2=BN_EPS,
            op0=mybir.AluOpType.mult, op1=mybir.AluOpType.add,
        )
        nc.vector.tensor_tensor(
            out=vareps[:], in0=vareps[:], in1=m2[:], op=mybir.AluOpType.subtract
        )
        rec1 = const.tile([128, 1], f32, name="rec1")
        nc.vector.reciprocal(out=rec1[:], in_=vareps[:])
        rsq = const.tile([128, 1], f32, name="rsq")
        nc.scalar.sqrt(out=rsq[:], in_=rec1[:])
        ab_sb = const.tile([128, 2], f32, name="ab_sb")
        nc.vector.tensor_tensor(
            out=ab_sb[:, 0:1], in0=rsq[:], in1=gb_sb[:, 0:1], op=mybir.AluOpType.mult
        )
        tmb = const.tile([128, 1], f32, name="tmb")
        nc.vector.tensor_tensor(
            out=tmb[:], in0=mean[:], in1=ab_sb[:, 0:1], op=mybir.AluOpType.mult
        )
        nc.vector.tensor_tensor(
            out=ab_sb[:, 1:2], in0=gb_sb[:, 1:2], in1=tmb[:], op=mybir.AluOpType.subtract
        )

        def bcast_col(col_ap, nm):
            tp = ps_misc.tile([128, 128], f32, tag="m")
            nc.tensor.transpose(out=tp[:1, :], in_=col_ap, identity=ident_sb[:])
            rowt = const.tile([1, 128], f32, name=f"rowt_{nm}")
            nc.vector.tensor_copy(out=rowt[:], in_=tp[:1, :])
            bc_ps = ps_misc.tile([128, 128], f32, tag="m")
            nc.tensor.matmul(out=bc_ps[:], lhsT=onesrow_sb[:], rhs=rowt[:], start=True, stop=True)
            bc = const.tile([128, 128], f32, name=f"bc_{nm}")
            nc.vector.tensor_copy(out=bc[:], in_=bc_ps[:])
            return bc

        a_bc = bcast_col(ab_sb[:, 0:1], "a")
        b_bc = bcast_col(ab_sb[:, 1:2], "b")

        # ---- final apply: y = relu(out*A + B) + x --------------------------
        xl = big.tile([128, NBLK * D], f32)
        nc.sync.dma_start(
            xl[:BLK, :], _strided(xloc_t, 0, [[D, BLK], [BLK * D, NBLK], [1, D]])
        )
        a_rep = _strided(a_bc[:], 0, [[a_bc[:].ap[0][0], BLK], [0, NBLK], [1, D]])
        b_rep = _strided(b_bc[:], 0, [[b_bc[:].ap[0][0], BLK], [0, NBLK], [1, D]])
        nc.vector.tensor_tensor(
            out=out_all[:BLK, :], in0=out_all[:BLK, :], in1=a_rep, op=mybir.AluOpType.mult
        )
        nc.vector.tensor_tensor(
            out=out_all[:BLK, :], in0=out_all[:BLK, :], in1=b_rep, op=mybir.AluOpType.add
        )
        nc.vector.tensor_scalar(
            out=out_all[:BLK, :], in0=out_all[:BLK, :], scalar1=0.0, scalar2=None,
            op0=mybir.AluOpType.max,
        )
        nc.vector.tensor_tensor(
            out=out_all[:BLK, :], in0=out_all[:BLK, :], in1=xl[:BLK, :],
            op=mybir.AluOpType.add,
        )
        nc.sync.dma_start(
            _strided(y_t, 0, [[D, BLK], [BLK * D, NBLK], [1, D]]), out_all[:BLK, :]
        )

    nc.compile()
    return nc


def _pack_idx(vals_by_seg, n_tiles, nblk):
    """Pack per-block index segments into the dma_gather int16 layout:
    idx i -> (partition i%16, col i//16), replicated across the 8 groups
    of 16 partitions.  Returns [128, nblk * n_tiles * 8] int16."""
    ncols = n_tiles * 8
    out = np.zeros((128, nblk * ncols), np.int16)
    for b, vals in enumerate(vals_by_seg):
        padded = np.zeros(n_tiles * 128, np.int16)
        padded[: len(vals)] = vals
        grid = padded.reshape(ncols, 16).T  # [16, ncols]
        out[:, b * ncols:(b + 1) * ncols] = np.tile(grid, (8, 1))
    return out


def prepare(x, edge_index, W, b, gamma, beta):
    x = np.asarray(x, np.float32)
    W = np.asarray(W, np.float32)
    gamma = np.asarray(gamma, np.float32)
    beta = np.asarray(beta, np.float32)
    N, D = x.shape
    assert N % CORES == 0
    NPC = N // CORES
    BLK = _pick_blk(NPC)
    NBLK = NPC // BLK
    NVT = N // BLK
    VB = 8
    while NVT % VB:
        VB -= 1
    SUP = 1
    for s in (5, 4, 3, 2):
        if NBLK % s == 0:
            SUP = s
            break
    SPL = min(SPLIT, N)

    row = np.asarray(edge_index[0]).astype(np.int64)
    col = np.asarray(edge_index[1]).astype(np.int64)
    rows = np.concatenate([row, np.arange(N, dtype=np.int64)])
    cols = np.concatenate([col, np.arange(N, dtype=np.int64)])
    # sort by (block, lo/hi) so each block's edges are lo-cols-first
    sort_key = (rows // BLK) * 2 + (cols >= SPL)
    order = np.argsort(sort_key, kind="stable")
    rs = rows[order]
    cs = cols[order]
    EE = rs.shape[0]

    NBLK_TOT = CORES * NBLK
    blk_of_edge = rs // BLK
    is_hi = cs >= SPL
    # counts per (block, lo/hi)
    seg_key = blk_of_edge * 2 + is_hi
    seg_cnt = np.bincount(seg_key, minlength=NBLK_TOT * 2)
    n_lo = seg_cnt[0::2]
    n_hi = seg_cnt[1::2]
    t_lo = max(1, int(np.ceil(n_lo.max() / 128)))
    t_hi = max(1, int(np.ceil(n_hi.max() / 128)))
    T = t_lo + t_hi

    # position within each (block, seg)
    seg_start = np.zeros(NBLK_TOT * 2 + 1, np.int64)
    np.cumsum(seg_cnt, out=seg_start[1:])
    pos_in_seg = np.arange(EE) - seg_start[seg_key]
    # tile index within the block (lo tiles then hi tiles)
    tile_in_blk = np.where(
        is_hi, t_lo + pos_in_seg // 128, pos_in_seg // 128
    )
    p_of = pos_in_seg % 128
    core_of = blk_of_edge // NBLK
    lblk = blk_of_edge % NBLK

    rel_arr = np.full((CORES, 128, NBLK * T), 200.0, np.float32)
    rel_arr[core_of, p_of, lblk * T + tile_in_blk] = (rs - blk_of_edge * BLK).astype(
        np.float32
    )
    rel_arr = rel_arr.astype(ml_dtypes.bfloat16)

    lo_idx = np.zeros((CORES, 128, NBLK * t_lo * 8), np.int16)
    hi_idx = np.zeros((CORES, 128, NBLK * t_hi * 8), np.int16)
    for k in range(CORES):
        lo_segs, hi_segs = [], []
        for lb in range(NBLK):
            gb = k * NBLK + lb
            lo_vals = cs[seg_start[2 * gb]:seg_start[2 * gb + 1]]
            hi_vals = cs[seg_start[2 * gb + 1]:seg_start[2 * gb + 2]] - SPL
            lo_segs.append(lo_vals.astype(np.int16))
            hi_segs.append(hi_vals.astype(np.int16))
        lo_idx[k] = _pack_idx(lo_segs, t_lo, NBLK)
        hi_idx[k] = _pack_idx(hi_segs, t_hi, NBLK)

    rp = np.searchsorted(np.sort(rows), np.arange(N + 1, dtype=np.int64)).astype(np.float32)
    RPF_PAD = NVT * BLK + 256
    rp_full = np.zeros(RPF_PAD, np.float32)
    rp_full[: N + 1] = rp
    RPL_PAD = NPC + 256
    rp_loc = np.zeros((CORES, RPL_PAD), np.float32)
    for k in range(CORES):
        rp_loc[k, : NPC + 1] = rp[k * NPC:(k + 1) * NPC + 1]

    XF_PAD = N + 128
    x_bf = np.zeros((XF_PAD, D), ml_dtypes.bfloat16)
    x_bf[:N] = x.astype(ml_dtypes.bfloat16)
    XL_PAD = NPC + 128
    x_loc = np.zeros((CORES, XL_PAD, D), np.float32)
    for k in range(CORES):
        x_loc[k, :NPC] = x[k * NPC:(k + 1) * NPC]

    iota = np.tile(np.arange(BLK, dtype=np.float32), (128, 1)).astype(
        ml_dtypes.bfloat16
    )

    in_maps = []
    for k in range(CORES):
        in_maps.append(
            {
                "x_bf": x_bf,
                "rp_full": rp_full,
                "rp_loc": rp_loc[k],
                "lo_idx": lo_idx[k],
                "hi_idx": hi_idx[k],
                "rel_arr": rel_arr[k],
                "x_loc": x_loc[k],
                "w_mat": W,
                "gamma": gamma,
                "beta": beta,
                "iota": iota,
            }
        )
    params = (N, D, NPC, BLK, NBLK, NVT, VB, t_lo, t_hi, SUP, SPL,
              XF_PAD, XL_PAD, RPF_PAD, RPL_PAD)
    return params, in_maps


def get_nc(params):
    if params not in _CACHE:
        _CACHE[params] = _build_nc(*params)
    return _CACHE[params]


def run(params, in_maps, trace=False, **kw):
    from concourse.bass_utils import run_bass_kernel_spmd

    nc = get_nc(params)
    res = run_bass_kernel_spmd(nc, in_maps, list(range(CORES)), trace=trace, **kw)
    y = np.concatenate([res.results[k]["y_out"] for k in range(CORES)], axis=0)
    return y.astype(np.float32), res


def kernel(x, edge_index, W, b, gamma, beta):
    params, in_maps = prepare(x, edge_index, W, b, gamma, beta)
    y, _ = run(params, in_maps)
    return y


# revision 10
# speedup vs baseline: 33.4524x; 33.4524x over previous
"""GCNConv(flow=target_to_source) + BatchNorm + ReLU + residual, on 8 trn2 NeuronCores.

Math: with self-loops appended to the edge list,
    deg[i]   = #{e : row[e] == i}
    dinv     = deg ** -0.5
    v        = dinv[:, None] * x                      (bf16 table in DRAM)
    S[i]     = sum_{e: row[e]=i} v[col[e]]            (dma_gather + onehot-matmul scatter)
    out      = dinv[:, None] * (S @ W)                (W commutes past the aggregation)
    y        = relu((out - mean) * rsqrt(var + eps) * gamma + beta) + x
(b cancels inside BatchNorm, so it is dropped.)

Sharding: nodes (rows) are split across 8 cores; edges are partitioned by
destination row so the scatter-add is core-local PSUM accumulation.  Each core
builds the full v table locally; BN statistics go through a [128,2] AllReduce.

dma_gather takes int16 indices, so the v table is addressed as two halves
(lo: rows < SPLIT, hi: rows >= SPLIT) and each block's edges are ordered
lo-cols-first.  Index buffers are packed in the HW layout: idx i at
(partition i%16, column i//16), replicated across the eight 16-partition
groups.
"""

import os
import sys

sys.path.insert(0, "/opt/trn_rl_repo")
os.environ.setdefault("MYCRO_LOCAL_CACHE", "1")

from contextlib import ExitStack

import ml_dtypes
import numpy as np

CORES = 8
BN_EPS = 1e-5
SPLIT = 32768
_CACHE: dict = {}


def _pick_blk(npc: int) -> int:
    for blk in range(125, 0, -1):
        if npc % blk == 0:
            return blk
    raise ValueError(npc)


def _strided(ap_src, offset_elems, dims):
    import concourse.bass as bass

    return bass.AP(ap_src.tensor, offset_elems, [list(d) for d in dims])


def _build_nc(N, D, NPC, BLK, NBLK, NVT, VB, t_lo, t_hi, SUP, SPL,
              XF_PAD, XL_PAD, RPF_PAD, RPL_PAD):
    from concourse import bacc, bass, mybir, tile
    from concourse.masks import make_identity

    f32 = mybir.dt.float32
    bf16 = mybir.dt.bfloat16
    i16 = mybir.dt.int16
    T = t_lo + t_hi

    nc = bacc.Bacc(
        "TRN2",
        target_bir_lowering=False,
        debug=False,
        enable_asserts=False,
        num_devices=CORES,
    )

    x_bf = nc.dram_tensor("x_bf", [XF_PAD, D], bf16, kind="ExternalInput").ap()
    rp_full = nc.dram_tensor("rp_full", [RPF_PAD], f32, kind="ExternalInput").ap()
    rp_loc = nc.dram_tensor("rp_loc", [RPL_PAD], f32, kind="ExternalInput").ap()
    lo_t = nc.dram_tensor("lo_idx", [128, NBLK * t_lo * 8], i16, kind="ExternalInput").ap()
    hi_t = nc.dram_tensor("hi_idx", [128, NBLK * t_hi * 8], i16, kind="ExternalInput").ap()
    rel_t = nc.dram_tensor("rel_arr", [128, NBLK * T], bf16, kind="ExternalInput").ap()
    xloc_t = nc.dram_tensor("x_loc", [XL_PAD, D], f32, kind="ExternalInput").ap()
    w_t = nc.dram_tensor("w_mat", [D, D], f32, kind="ExternalInput").ap()
    gamma_t = nc.dram_tensor("gamma", [D], f32, kind="ExternalInput").ap()
    beta_t = nc.dram_tensor("beta", [D], f32, kind="ExternalInput").ap()
    iota_t = nc.dram_tensor("iota", [128, BLK], bf16, kind="ExternalInput").ap()
    y_t = nc.dram_tensor("y_out", [NPC, D], f32, kind="ExternalOutput").ap()

    with tile.TileContext(nc) as tc, ExitStack() as ctx:
        const = ctx.enter_context(tc.tile_pool(name="const", bufs=1))
        vstage = ctx.enter_context(tc.tile_pool(name="vstage", bufs=2))
        gath = ctx.enter_context(tc.tile_pool(name="gath", bufs=2))
        ohp = ctx.enter_context(tc.tile_pool(name="ohp", bufs=3))
        evp = ctx.enter_context(tc.tile_pool(name="evp", bufs=2))
        big = ctx.enter_context(tc.tile_pool(name="big", bufs=1))
        ps_main = ctx.enter_context(tc.tile_pool(name="ps_main", bufs=2, space="PSUM"))
        ps_stat = ctx.enter_context(tc.tile_pool(name="ps_stat", bufs=1, space="PSUM"))
        ps_misc = ctx.enter_context(tc.tile_pool(name="ps_misc", bufs=2, space="PSUM"))
        dram = ctx.enter_context(tc.tile_pool(name="dram", bufs=1, space="DRAM"))

        # ---- constants -----------------------------------------------------
        w_sb = const.tile([D, D], f32)
        nc.sync.dma_start(w_sb[:], w_t[:])
        iota_sb = const.tile([128, BLK], bf16)
        nc.sync.dma_start(iota_sb[:], iota_t[:])
        lo_sb = const.tile([128, NBLK * t_lo * 8], i16)
        nc.sync.dma_start(lo_sb[:], lo_t[:])
        hi_sb = const.tile([128, NBLK * t_hi * 8], i16)
        nc.sync.dma_start(hi_sb[:], hi_t[:])
        rel_sb = const.tile([128, NBLK * T], bf16)
        nc.sync.dma_start(rel_sb[:], rel_t[:])
        ones_sb = const.tile([128, 1], f32)
        nc.vector.memset(ones_sb[:], 1.0)
        onesrow_sb = const.tile([1, 128], f32)
        nc.vector.memset(onesrow_sb[:], 1.0)
        gb_sb = const.tile([128, 2], f32)
        nc.sync.dma_start(gb_sb[:, 0:1], gamma_t[:, None])
        nc.sync.dma_start(gb_sb[:, 1:2], beta_t[:, None])
        ident_sb = const.tile([128, 128], f32)
        make_identity(nc, ident_sb[:])

        # ---- dinv ----------------------------------------------------------
        def build_dinv(rp_ap, nt):
            rpa = vstage.tile([128, nt], f32, tag="rpa")
            rpb = vstage.tile([128, nt], f32, tag="rpb")
            nc.sync.dma_start(rpa[:], _strided(rp_ap, 0, [[1, 128], [BLK, nt]]))
            nc.sync.dma_start(rpb[:], _strided(rp_ap, 1, [[1, 128], [BLK, nt]]))
            deg = vstage.tile([128, nt], f32, tag="deg")
            nc.vector.tensor_tensor(
                out=deg[:BLK, :], in0=rpb[:BLK, :], in1=rpa[:BLK, :],
                op=mybir.AluOpType.subtract,
            )
            rec = vstage.tile([128, nt], f32, tag="rec")
            nc.vector.reciprocal(out=rec[:BLK, :], in_=deg[:BLK, :])
            dv = const.tile([128, nt], f32, name=f"dinv{nt}")
            nc.scalar.sqrt(out=dv[:BLK, :], in_=rec[:BLK, :])
            return dv

        dinvl = build_dinv(rp_loc, NBLK)

        # dinv in flat layout: partition p owns rows [p*RPP, (p+1)*RPP)
        RPP = XF_PAD // 128
        rpa_f = vstage.tile([128, RPP], f32, tag="rpa")
        rpb_f = vstage.tile([128, RPP], f32, tag="rpb")
        nc.sync.dma_start(rpa_f[:], _strided(rp_full, 0, [[RPP, 128], [1, RPP]]))
        nc.sync.dma_start(rpb_f[:], _strided(rp_full, 1, [[RPP, 128], [1, RPP]]))
        deg_f = vstage.tile([128, RPP], f32, tag="deg")
        nc.vector.tensor_tensor(
            out=deg_f[:], in0=rpb_f[:], in1=rpa_f[:], op=mybir.AluOpType.subtract
        )
        rec_f = vstage.tile([128, RPP], f32, tag="rec")
        nc.vector.reciprocal(out=rec_f[:], in_=deg_f[:])
        dinvf = const.tile([128, RPP], f32, name="dinvf")
        nc.scalar.sqrt(out=dinvf[:], in_=rec_f[:])

        # ---- v table: v = dinv * x (bf16, in DRAM), flat stripes -----------
        v_dram = dram.tile([XF_PAD, D], bf16)
        VCH = 28
        while RPP % VCH and VCH > 1:
            VCH -= 1
        row0 = 0
        while row0 < RPP:
            ch = min(VCH, RPP - row0)
            xs = vstage.tile([128, VCH * D], bf16, tag="xs")
            nc.sync.dma_start(
                xs[:, : ch * D],
                _strided(x_bf, row0 * D, [[RPP * D, 128], [1, ch * D]]),
            )
            vv = vstage.tile([128, VCH * D], bf16, tag="vv")
            dinv_rep = _strided(
                dinvf[:], row0, [[dinvf[:].ap[0][0], 128], [1, ch], [0, D]]
            )
            nc.vector.tensor_tensor(
                out=vv[:, : ch * D], in0=xs[:, : ch * D], in1=dinv_rep,
                op=mybir.AluOpType.mult,
            )
            nc.sync.dma_start(
                _strided(v_dram[:], row0 * D, [[RPP * D, 128], [1, ch * D]]),
                vv[:, : ch * D],
            )
            row0 += ch

        # ---- main loop: SUP blocks per gather chunk ------------------------
        out_all = big.tile([128, NBLK * D], f32)
        s1 = ps_stat.tile([128, 1], f32, tag="s1")
        s2 = ps_stat.tile([128, 1], f32, tag="s2")
        for c0 in range(0, NBLK, SUP):
            g = gath.tile([128, SUP * T, D], bf16)
            nc.gpsimd.dma_gather(
                g[:, 0:SUP * t_lo, :],
                v_dram[0:SPL, :],
                lo_sb[:, c0 * t_lo * 8:(c0 + SUP) * t_lo * 8],
                SUP * t_lo * 128,
                SUP * t_lo * 128,
                D,
                single_packet=False,
            )
            nc.gpsimd.dma_gather(
                g[:, SUP * t_lo:SUP * T, :],
                v_dram[SPL:XF_PAD, :],
                hi_sb[:, c0 * t_hi * 8:(c0 + SUP) * t_hi * 8],
                SUP * t_hi * 128,
                SUP * t_hi * 128,
                D,
                single_packet=False,
            )
            for j in range(SUP):
                blk = c0 + j
                oh = ohp.tile([128, T, BLK], bf16)
                iota_rep = _strided(
                    iota_sb[:], 0, [list(iota_sb[:].ap[0]), [0, T], [1, BLK]]
                )
                rel_rep = _strided(
                    rel_sb[:], blk * T, [list(rel_sb[:].ap[0]), [1, T], [0, BLK]]
                )
                nc.vector.tensor_tensor(
                    out=oh[:], in0=iota_rep, in1=rel_rep, op=mybir.AluOpType.is_equal
                )
                st = ps_main.tile([128, BLK], f32, tag="st")
                for t in range(T):
                    if t < t_lo:
                        src = g[:, j * t_lo + t, :]
                    else:
                        src = g[:, SUP * t_lo + j * t_hi + (t - t_lo), :]
                    nc.tensor.matmul(
                        out=st[:], lhsT=src, rhs=oh[:, t, :],
                        start=(t == 0), stop=(t == T - 1),
                    )
                stb = evp.tile([128, BLK], f32, tag="stb")
                nc.vector.tensor_copy(out=stb[:], in_=st[:])
                ow = ps_main.tile([BLK, D], f32, tag="ow")
                nc.tensor.matmul(out=ow[:], lhsT=stb[:], rhs=w_sb[:], start=True, stop=True)
                oslice = out_all[:BLK, blk * D:(blk + 1) * D]
                nc.vector.tensor_scalar(
                    out=oslice, in0=ow[:], scalar1=dinvl[:BLK, blk:blk + 1],
                    scalar2=None, op0=mybir.AluOpType.mult,
                )
                sq_s = evp.tile([128, D], f32, tag="sq")
                nc.vector.tensor_tensor(
                    out=sq_s[:BLK, :], in0=oslice, in1=oslice, op=mybir.AluOpType.mult
                )
                nc.tensor.matmul(
                    out=s1[:], lhsT=oslice, rhs=ones_sb[:BLK, :],
                    start=(blk == 0), stop=(blk == NBLK - 1),
                )
                nc.tensor.matmul(
                    out=s2[:], lhsT=sq_s[:BLK, :], rhs=ones_sb[:BLK, :],
                    start=(blk == 0), stop=(blk == NBLK - 1),
                )

        # ---- BN stats AllReduce + affine params ----------------------------
        stat_sb = const.tile([128, 2], f32, name="stat_sb")
        nc.vector.tensor_copy(out=stat_sb[:, 0:1], in_=s1[:])
        nc.vector.tensor_copy(out=stat_sb[:, 1:2], in_=s2[:])
        cc_in = dram.tile([128, 2], f32)
        cc_out = dram.tile([128, 2], f32, addr_space="Shared")
        nc.sync.dma_start(cc_in[:], stat_sb[:])
        nc.gpsimd.collective_compute(
            "AllReduce",
            mybir.AluOpType.add,
            replica_groups=[list(range(CORES))],
            ins=[cc_in.opt()],
            outs=[cc_out.opt()],
        )
        statg = const.tile([128, 2], f32, name="statg")
        nc.sync.dma_start(statg[:], cc_out[:])

        invn = 1.0 / float(N)
        mean = const.tile([128, 1], f32, name="mean")
        nc.vector.tensor_scalar(
            out=mean[:], in0=statg[:, 0:1], scalar1=invn, scalar2=None,
            op0=mybir.AluOpType.mult,
        )
        vareps = const.tile([128, 1], f32, name="vareps")
        m2 = const.tile([128, 1], f32, name="m2")
        nc.vector.tensor_tensor(out=m2[:], in0=mean[:], in1=mean[:], op=mybir.AluOpType.mult)
        nc.vector.tensor_scalar(
            out=vareps[:], in0=statg[:, 1:2], scalar1=invn, scalar2=BN_EPS,
            op0=mybir.AluOpType.mult, op1=mybir.AluOpType.add,
        )
        nc.vector.tensor_tensor(
            out=vareps[:], in0=vareps[:], in1=m2[:], op=mybir.AluOpType.subtract
        )
        rec1 = const.tile([128, 1], f32, name="rec1")
        nc.vector.reciprocal(out=rec1[:], in_=vareps[:])
        rsq = const.tile([128, 1], f32, name="rsq")
        nc.scalar.sqrt(out=rsq[:], in_=rec1[:])
        ab_sb = const.tile([128, 2], f32, name="ab_sb")
        nc.vector.tensor_tensor(
            out=ab_sb[:, 0:1], in0=rsq[:], in1=gb_sb[:, 0:1], op=mybir.AluOpType.mult
        )
        tmb = const.tile([128, 1], f32, name="tmb")
        nc.vector.tensor_tensor(
            out=tmb[:], in0=mean[:], in1=ab_sb[:, 0:1], op=mybir.AluOpType.mult
        )
        nc.vector.tensor_tensor(
            out=ab_sb[:, 1:2], in0=gb_sb[:, 1:2], in1=tmb[:], op=mybir.AluOpType.subtract
        )

        def bcast_col(col_ap, nm):
            tp = ps_misc.tile([128, 128], f32, tag="m")
            nc.tensor.transpose(out=tp[:1, :], in_=col_ap, identity=ident_sb[:])
            rowt = const.tile([1, 128], f32, name=f"rowt_{nm}")
            nc.vector.tensor_copy(out=rowt[:], in_=tp[:1, :])
            bc_ps = ps_misc.tile([128, 128], f32, tag="m")
            nc.tensor.matmul(out=bc_ps[:], lhsT=onesrow_sb[:], rhs=rowt[:], start=True, stop=True)
            bc = const.tile([128, 128], f32, name=f"bc_{nm}")
            nc.vector.tensor_copy(out=bc[:], in_=bc_ps[:])
            return bc

        a_bc = bcast_col(ab_sb[:, 0:1], "a")
        b_bc = bcast_col(ab_sb[:, 1:2], "b")

        # ---- final apply: y = relu(out*A + B) + x --------------------------
        xl = big.tile([128, NBLK * D], f32)
        nc.sync.dma_start(
            xl[:BLK, :], _strided(xloc_t, 0, [[D, BLK], [BLK * D, NBLK], [1, D]])
        )
        a_rep = _strided(a_bc[:], 0, [[a_bc[:].ap[0][0], BLK], [0, NBLK], [1, D]])
        b_rep = _strided(b_bc[:], 0, [[b_bc[:].ap[0][0], BLK], [0, NBLK], [1, D]])
        nc.vector.tensor_tensor(
            out=out_all[:BLK, :], in0=out_all[:BLK, :], in1=a_rep, op=mybir.AluOpType.mult
        )
        nc.vector.tensor_tensor(
            out=out_all[:BLK, :], in0=out_all[:BLK, :], in1=b_rep, op=mybir.AluOpType.add
        )
        nc.vector.tensor_scalar(
            out=out_all[:BLK, :], in0=out_all[:BLK, :], scalar1=0.0, scalar2=None,
            op0=mybir.AluOpType.max,
        )
        nc.vector.tensor_tensor(
            out=out_all[:BLK, :], in0=out_all[:BLK, :], in1=xl[:BLK, :],
            op=mybir.AluOpType.add,
        )
        nc.sync.dma_start(
            _strided(y_t, 0, [[D, BLK], [BLK * D, NBLK], [1, D]]), out_all[:BLK, :]
        )

    nc.compile()
    return nc


def _pack_idx(vals_by_seg, n_tiles, nblk):
    """Pack per-block index segments into the dma_gather int16 layout:
    idx i -> (partition i%16, col i//16), replicated across the 8 groups
    of 16 partitions.  Returns [128, nblk * n_tiles * 8] int16."""
    ncols = n_tiles * 8
    out = np.zeros((128, nblk * ncols), np.int16)
    for b, vals in enumerate(vals_by_seg):
        padded = np.zeros(n_tiles * 128, np.int16)
        padded[: len(vals)] = vals
        grid = padded.reshape(ncols, 16).T  # [16, ncols]
        out[:, b * ncols:(b + 1) * ncols] = np.tile(grid, (8, 1))
    return out


def prepare(x, edge_index, W, b, gamma, beta):
    x = np.asarray(x, np.float32)
    W = np.asarray(W, np.float32)
    gamma = np.asarray(gamma, np.float32)
    beta = np.asarray(beta, np.float32)
    N, D = x.shape
    assert N % CORES == 0
    NPC = N // CORES
    BLK = _pick_blk(NPC)
    NBLK = NPC // BLK
    NVT = N // BLK
    VB = 8
    while NVT % VB:
        VB -= 1
    SUP = 1
    for s in (5, 4, 3, 2):
        if NBLK % s == 0:
            SUP = s
            break
    SPL = min(SPLIT, N)

    row = np.asarray(edge_index[0]).astype(np.int64)
    col = np.asarray(edge_index[1]).astype(np.int64)
    rows = np.concatenate([row, np.arange(N, dtype=np.int64)])
    cols = np.concatenate([col, np.arange(N, dtype=np.int64)])
    # sort by (block, lo/hi) so each block's edges are lo-cols-first
    sort_key = (rows // BLK) * 2 + (cols >= SPL)
    order = np.argsort(sort_key, kind="stable")
    rs = rows[order]
    cs = cols[order]
    EE = rs.shape[0]

    NBLK_TOT = CORES * NBLK
    blk_of_edge = rs // BLK
    is_hi = cs >= SPL
    # counts per (block, lo/hi)
    seg_key = blk_of_edge * 2 + is_hi
    seg_cnt = np.bincount(seg_key, minlength=NBLK_TOT * 2)
    n_lo = seg_cnt[0::2]
    n_hi = seg_cnt[1::2]
    t_lo = max(1, int(np.ceil(n_lo.max() / 128)))
    t_hi = max(1, int(np.ceil(n_hi.max() / 128)))
    T = t_lo + t_hi

    # position within each (block, seg)
    seg_start = np.zeros(NBLK_TOT * 2 + 1, np.int64)
    np.cumsum(seg_cnt, out=seg_start[1:])
    pos_in_seg = np.arange(EE) - seg_start[seg_key]
    # tile index within the block (lo tiles then hi tiles)
    tile_in_blk = np.where(
        is_hi, t_lo + pos_in_seg // 128, pos_in_seg // 128
    )
    p_of = pos_in_seg % 128
    core_of = blk_of_edge // NBLK
    lblk = blk_of_edge % NBLK

    rel_arr = np.full((CORES, 128, NBLK * T), 200.0, np.float32)
    rel_arr[core_of, p_of, lblk * T + tile_in_blk] = (rs - blk_of_edge * BLK).astype(
        np.float32
    )
    rel_arr = rel_arr.astype(ml_dtypes.bfloat16)

    lo_idx = np.zeros((CORES, 128, NBLK * t_lo * 8), np.int16)
    hi_idx = np.zeros((CORES, 128, NBLK * t_hi * 8), np.int16)
    for k in range(CORES):
        lo_segs, hi_segs = [], []
        for lb in range(NBLK):
            gb = k * NBLK + lb
            lo_vals = cs[seg_start[2 * gb]:seg_start[2 * gb + 1]]
            hi_vals = cs[seg_start[2 * gb + 1]:seg_start[2 * gb + 2]] - SPL
            lo_segs.append(lo_vals.astype(np.int16))
            hi_segs.append(hi_vals.astype(np.int16))
        lo_idx[k] = _pack_idx(lo_segs, t_lo, NBLK)
        hi_idx[k] = _pack_idx(hi_segs, t_hi, NBLK)

    rp = np.searchsorted(np.sort(rows), np.arange(N + 1, dtype=np.int64)).astype(np.float32)
    RPF_PAD = ((N + 128 + 127) // 128) * 128 + 8
    rp_full = np.zeros(RPF_PAD, np.float32)
    rp_full[: N + 1] = rp
    # pad rows get degree 1 so dinv stays finite (v pad rows = 0 * 1 = 0)
    rp_full[N + 1:] = rp[N] + np.arange(1, RPF_PAD - N, dtype=np.float32)
    RPL_PAD = NPC + 256
    rp_loc = np.zeros((CORES, RPL_PAD), np.float32)
    for k in range(CORES):
        rp_loc[k, : NPC + 1] = rp[k * NPC:(k + 1) * NPC + 1]

    XF_PAD = ((N + 128 + 127) // 128) * 128
    x_bf = np.zeros((XF_PAD, D), ml_dtypes.bfloat16)
    x_bf[:N] = x.astype(ml_dtypes.bfloat16)
    XL_PAD = NPC + 128
    x_loc = np.zeros((CORES, XL_PAD, D), np.float32)
    for k in range(CORES):
        x_loc[k, :NPC] = x[k * NPC:(k + 1) * NPC]

    iota = np.tile(np.arange(BLK, dtype=np.float32), (128, 1)).astype(
        ml_dtypes.bfloat16
    )

    in_maps = []
    for k in range(CORES):
        in_maps.append(
            {
                "x_bf": x_bf,
                "rp_full": rp_full,
                "rp_loc": rp_loc[k],
                "lo_idx": lo_idx[k],
                "hi_idx": hi_idx[k],
                "rel_arr": rel_arr[k],
                "x_loc": x_loc[k],
                "w_mat": W,
                "gamma": gamma,
                "beta": beta,
                "iota": iota,
            }
        )
    params = (N, D, NPC, BLK, NBLK, NVT, VB, t_lo, t_hi, SUP, SPL,
              XF_PAD, XL_PAD, RPF_PAD, RPL_PAD)
    return params, in_maps


def get_nc(params):
    if params not in _CACHE:
        _CACHE[params] = _build_nc(*params)
    return _CACHE[params]


def run(params, in_maps, trace=False, **kw):
    from concourse.bass_utils import run_bass_kernel_spmd

    nc = get_nc(params)
    res = run_bass_kernel_spmd(nc, in_maps, list(range(CORES)), trace=trace, **kw)
    y = np.concatenate([res.results[k]["y_out"] for k in range(CORES)], axis=0)
    return y.astype(np.float32), res


def kernel(x, edge_index, W, b, gamma, beta):
    params, in_maps = prepare(x, edge_index, W, b, gamma, beta)
    y, _ = run(params, in_maps)
    return y
